# revision 14
# baseline (speedup 1.0000x reference)
"""MoE patch encoder on 8 Trainium2 NeuronCores.

Strategy:
- Host: patchify, run the (tiny) router in fp32 to build a dispatch plan:
  patches sorted by expert, cores assigned (expert, patch-range, out-range)
  jobs balancing HBM bytes and PE flops. Device re-computes the router in
  fp32 for the probs/expert_id outputs.
- Device (one SPMD program, per-core arms via an If tree on partition id):
  each core routes its 128 patches (fp32) and runs its expert jobs in bf16
  (weights cast on host): x -> attn(v) -> layernorm -> p1(relu) -> p2(tanh),
  all in transposed [feature, patch] layout so biases are per-partition.
- Host: scatter job outputs back to the full [1024, 120, 4, 4] latents.
"""
import sys
import os
import numpy as np

for _p in ("/opt/trn_rl_repo", os.path.dirname(os.path.abspath(__file__))):
    if _p not in sys.path:
        sys.path.insert(0, _p)

import ml_dtypes
import concourse.bass as bass
import concourse.mybir as mybir
import concourse.tile as tile
from concourse.bass_utils import run_bass_kernel_spmd
from contextlib import ExitStack

F32 = mybir.dt.float32
BF16 = mybir.dt.bfloat16
I32 = mybir.dt.int32
U32 = mybir.dt.uint32
BF = ml_dtypes.bfloat16

PS = 16
EMB = 256
E = 4
CL = [8, 16, 32, 64]
OUT_E = [c * 16 for c in CL]  # [128, 256, 512, 1024]
P1 = 16384
H = W = 512
NPATCH = (H // PS) * (W // PS)  # 1024
NCORES = 8
FEAT = 3 * PS * PS  # 768
KC_FEAT = FEAT // 128  # 6
MGRP = 32  # p1/p2 m-chunks per DMA group

NEG_BIG = -1.0e30


def _cap_sync_waits(nc, max_waits=1):
    """walrus here rejects >N waits per instruction; split onto nops."""
    def fix_block(bb):
        new_insts = []
        for inst in bb.instructions:
            si = getattr(inst, "sync_info", None)
            if si is not None and si.on_wait and len(si.on_wait) > max_waits:
                waits = list(si.on_wait)
                extra, keep = waits[:-max_waits], waits[-max_waits:]
                for i in range(0, len(extra), max_waits):
                    chunk = extra[i : i + max_waits]
                    nop = mybir.InstNoOp(
                        name=f"{inst.name}-ws{i}",
                        sync_info=mybir.SyncInfo(on_wait=chunk, on_update=[]),
                        engine=inst.engine,
                        bass_nofuse=True,
                    )
                    new_insts.append(nop)
                si.on_wait = keep
            new_insts.append(inst)
        bb.instructions = new_insts

    def walk(blocks):
        for b in blocks:
            if hasattr(b, "instructions"):
                fix_block(b)
            inner = getattr(b, "blocks", None)
            if inner:
                walk(inner)

    for f in nc.m.functions:
        walk(f.blocks)


def _patchify(image):
    img = np.asarray(image, np.float32)[0]
    gh, gw = H // PS, W // PS
    return (
        img.reshape(3, gh, PS, gw, PS)
        .transpose(1, 3, 0, 2, 4)
        .reshape(gh * gw, FEAT)
    )


def _split_even(total, parts):
    q, r = divmod(total, parts)
    return [q + (1 if i < r else 0) for i in range(parts)]


def _expert_split(e, n_e, g):
    """Best (per-core time, per-core job lists) for expert e on g cores."""
    NJ = OUT_E[e] // 128
    best = None
    for mm in range(1, g + 1):
        if g % mm or mm > NJ:
            continue
        pm = g // mm
        jgrp = _split_even(NJ, mm)
        nblk = -(-n_e // pm)
        pjobs = -(-nblk // 512)
        cores = []
        tmax = 0.0
        for mi in range(mm):
            j0 = sum(jgrp[:mi])
            gj = jgrp[mi]
            mjobs = -(-gj // 4)  # psum limit: 4 j-chunks per job
            jsz = _split_even(gj, mjobs)
            for pi in range(pm):
                p_off = pi * nblk
                n_here = max(0, min(n_e - p_off, nblk))
                jl = []
                for pj in range(pjobs):
                    po = p_off + pj * 512
                    nr = max(0, min(n_here - pj * 512, 512))
                    if nr == 0:
                        continue
                    jj = j0
                    for js in jsz:
                        jl.append((po, nr, jj, js))
                        jj += js
                cores.append(jl)
                b = sum(P1 * EMB * 2 + P1 * (js_ * 128) * 2
                        for (_, _, _, js_) in jl) + n_here * FEAT * 2 + 2.0e6
                f = sum(nr_ * 2.0 * (EMB * P1 + P1 * 128 * js_ + FEAT * EMB
                                     + 2 * EMB * EMB)
                        for (_, nr_, _, js_) in jl)
                t = max(b / 340e3, f / 70e3) + 8e3 + len(jl) * 4e3
                tmax = max(tmax, t)
        if best is None or tmax < best[0]:
            best = (tmax, cores)
    return best


def _plan(counts):
    """Assign 8 cores to (expert, patch-range, out-range) jobs."""
    seg = np.concatenate([[0], np.cumsum(counts)])
    active = [e for e in range(E) if counts[e] > 0]

    def compositions(total, parts):
        if parts == 1:
            yield (total,)
            return
        for first in range(1, total - parts + 2):
            for rest in compositions(total - first, parts - 1):
                yield (first,) + rest

    best = None
    for alloc in compositions(NCORES, len(active)):
        tmax = 0.0
        detail = []
        for e, g in zip(active, alloc):
            r = _expert_split(e, int(counts[e]), g)
            if r is None:
                tmax = float("inf")
                break
            tmax = max(tmax, r[0])
            detail.append((e, r[1]))
        if best is None or tmax < best[0]:
            best = (tmax, detail)

    jobs = [[] for _ in range(NCORES)]
    core = 0
    for e, cores in best[1]:
        for jl in cores:
            for (po, nr, j0, njc) in jl:
                n_pad = min(512, max(16, -(-nr // 8) * 8))
                jobs[core].append(dict(
                    e=e, p0=int(seg[e] + po), n_real=int(nr), n_pad=int(n_pad),
                    m0=int(j0 * 128), msz=int(njc * 128)))
            core += 1
    return jobs


# bias10 column map (per expert, appended to fb array):
#   0,1: pe_b k2=0,1   2,3: vb   4,5: ob   6,7: ln_g   8,9: ln_b
BIAS_COLS = {"peb": 0, "vb": 2, "ob": 4, "lng": 6, "lnb": 8}


def _build(jobs):
    nc = bass.Bass()
    d = {}

    def din(name, shape, dt):
        d[name] = nc.dram_tensor(name, list(shape), dt, kind="ExternalInput")
        return d[name]

    def dout(name, shape, dt):
        d[name] = nc.dram_tensor(name, list(shape), dt, kind="ExternalOutput")
        return d[name]

    for c in range(NCORES):
        din(f"featr{c}", (128, KC_FEAT, 128), F32)
    din("rwpack", (128, KC_FEAT * EMB + 2 * E), F32)
    din("rb1", (128, 2), F32)
    din("rb2row", (1, E), F32)

    experts_used = sorted({jb["e"] for cj in jobs for jb in cj})
    for e in experts_used:
        din(f"smw{e}", (128, 10, EMB), BF16)
        din(f"p1wT{e}", (P1 // (MGRP * 128), 128, 2, MGRP * 128), BF16)
        din(f"fb{e}", (128, 128 + OUT_E[e] // 128 + 10), F32)
    for c in range(NCORES):
        for ji, jb in enumerate(jobs[c]):
            din(f"feat_c{c}_j{ji}", (128, KC_FEAT, jb["n_pad"]), BF16)
            din(f"p2w_c{c}_j{ji}", (128, 128, jb["msz"]), BF16)

    dout("probs", (128, E), F32)
    dout("eid", (128, 1), I32)
    for c in range(NCORES):
        for ji, jb in enumerate(jobs[c]):
            dout(f"out_c{c}_j{ji}", (jb["msz"], jb["n_pad"]), F32)

    with tile.TileContext(nc) as tc, ExitStack() as ctx:
        const = ctx.enter_context(tc.tile_pool(name="const", bufs=1))
        work = ctx.enter_context(tc.tile_pool(name="work", bufs=2))
        wstream = ctx.enter_context(tc.tile_pool(name="wstream", bufs=2))
        wres = ctx.enter_context(tc.tile_pool(name="wres", bufs=1))
        l1pool = ctx.enter_context(tc.tile_pool(name="l1pool", bufs=3))
        psum = ctx.enter_context(tc.tile_pool(name="psum", bufs=2, space="PSUM"))
        psout = ctx.enter_context(tc.tile_pool(name="psout", bufs=1, space="PSUM"))

        ones_col = const.tile([128, 1], F32)
        nc.vector.memset(ones_col, 1.0)
        ones_row = const.tile([1, 128], F32)
        nc.vector.memset(ones_row, 1.0)
        eps_sb = const.tile([1, 1], F32)
        nc.vector.memset(eps_sb, 1.0e-5)

        pid = nc.partition_id()

        def emit_arm(c):
            with tc.high_priority():
                _emit_router(nc, tc, d, c, work, psum, ones_row)
            loaded = {}
            for ji, jb in enumerate(jobs[c]):
                e = jb["e"]
                if e not in loaded:
                    smw = work.tile([128, 10, EMB], BF16, name=f"smw_{c}_{e}",
                                    tag="smw")
                    nc.sync.dma_start(out=smw, in_=d[f"smw{e}"][:])
                    fb = work.tile([128, 128 + OUT_E[e] // 128 + 10], F32,
                                   name=f"fb_{c}_{e}", tag="fb")
                    nc.sync.dma_start(out=fb, in_=d[f"fb{e}"][:])
                    loaded[e] = (smw, fb)
                smw, fb = loaded[e]
                _emit_job(nc, tc, d, c, ji, jb, smw, fb, work, wstream, wres,
                          l1pool, psum, psout, ones_col, ones_row, eps_sb)

        def emit_tree(lo, hi):
            if hi - lo == 1:
                emit_arm(lo)
                return
            mid = (lo + hi) // 2
            with tc.If(pid < mid) as cmp:
                emit_tree(lo, mid)
            with cmp.Else():
                emit_tree(mid, hi)

        emit_tree(0, NCORES)

    _cap_sync_waits(nc, max_waits=1)
    return nc


def _emit_router(nc, tc, d, c, work, psum, ones_row):
    featr = work.tile([128, KC_FEAT, 128], F32, tag="featr")
    nc.sync.dma_start(out=featr, in_=d[f"featr{c}"][:])
    rwp = work.tile([128, KC_FEAT * EMB + 2 * E], F32, tag="rwp")
    nc.sync.dma_start(out=rwp, in_=d["rwpack"][:])
    rb1_sb = work.tile([128, 2], F32, tag="rb1sb")
    nc.sync.dma_start(out=rb1_sb, in_=d["rb1"][:])
    rb2_sb = work.tile([1, E], F32, tag="rb2sb")
    nc.sync.dma_start(out=rb2_sb, in_=d["rb2row"][:])

    hrT = work.tile([128, 2, 128], F32, tag="hrT")
    for m2 in range(2):
        ps = psum.tile([128, 128], F32, tag="ps_big")
        for k in range(KC_FEAT):
            nc.tensor.matmul(
                ps, rwp[:, k * EMB + m2 * 128 : k * EMB + (m2 + 1) * 128],
                featr[:, k], start=(k == 0), stop=(k == KC_FEAT - 1))
        nc.vector.tensor_scalar(out=hrT[:, m2], in0=ps,
                                scalar1=rb1_sb[:, m2 : m2 + 1], scalar2=0.0,
                                op0=mybir.AluOpType.add, op1=mybir.AluOpType.max)
    lg_ps = psum.tile([128, E], F32, tag="ps_big")
    for k2 in range(2):
        nc.tensor.matmul(
            lg_ps, hrT[:, k2],
            rwp[:, KC_FEAT * EMB + k2 * E : KC_FEAT * EMB + (k2 + 1) * E],
            start=(k2 == 0), stop=False)
    nc.tensor.matmul(lg_ps, ones_row, rb2_sb, start=False, stop=True)

    lg = work.tile([128, 8], F32, tag="lg")
    nc.vector.memset(lg, NEG_BIG)
    nc.vector.tensor_copy(lg[:, 0:E], lg_ps)
    mx = work.tile([128, 1], F32, tag="mx")
    nc.vector.tensor_reduce(out=mx, in_=lg[:, 0:E], axis=mybir.AxisListType.X,
                            op=mybir.AluOpType.max)
    negmx = work.tile([128, 1], F32, tag="negmx")
    nc.vector.tensor_scalar_mul(negmx, mx, -1.0)
    expt = work.tile([128, E], F32, tag="expt")
    sume = work.tile([128, 1], F32, tag="sume")
    nc.scalar.activation(out=expt, in_=lg[:, 0:E],
                         func=mybir.ActivationFunctionType.Exp,
                         bias=negmx[:, 0:1], scale=1.0, accum_out=sume)
    rec = work.tile([128, 1], F32, tag="rec")
    nc.vector.reciprocal(rec, sume)
    probs = work.tile([128, E], F32, tag="probs")
    nc.vector.tensor_scalar_mul(probs, expt, rec[:, 0:1])
    nc.sync.dma_start(out=d["probs"][:], in_=probs)

    top8 = work.tile([128, 8], F32, tag="top8")
    nc.vector.max(top8, lg)
    idx8 = work.tile([128, 8], U32, tag="idx8")
    nc.vector.max_index(idx8, top8, lg)
    eid = work.tile([128, 1], I32, tag="eid")
    nc.vector.tensor_copy(eid, idx8[:, 0:1])
    nc.sync.dma_start(out=d["eid"][:], in_=eid)


def _emit_job(nc, tc, d, c, ji, jb, smw, fb, work, wstream, wres, l1pool, psum,
              psout, ones_col, ones_row, eps_sb):
    e, n, m0, msz = jb["e"], jb["n_pad"], jb["m0"], jb["msz"]
    AT = mybir.AluOpType
    NJ_E = OUT_E[e] // 128

    def bias(nm, m2):
        col = 128 + NJ_E + BIAS_COLS[nm] + m2
        return fb[:, col : col + 1]

    feats = work.tile([128, KC_FEAT, n], BF16, tag="feats")
    nc.sync.dma_start(out=feats, in_=d[f"feat_c{c}_j{ji}"][:])

    # x.T = pe_w @ feat.T (+peb); bf16 for matmuls, f32 for the residual
    xT = work.tile([128, 2, n], BF16, tag="xT")
    xT32 = work.tile([128, 2, n], F32, tag="xT32")
    for m2 in range(2):
        ps = psum.tile([128, n], F32, tag="ps_big")
        for k in range(KC_FEAT):
            nc.tensor.matmul(ps, smw[:, k, m2 * 128 : (m2 + 1) * 128],
                             feats[:, k], start=(k == 0), stop=(k == KC_FEAT - 1))
        nc.vector.tensor_scalar_add(xT[:, m2], ps, bias("peb", m2))
        nc.vector.tensor_scalar_add(xT32[:, m2], ps, bias("peb", m2))
    # v.T = wv @ x.T (+vb)
    vT = work.tile([128, 2, n], BF16, tag="vT")
    for m2 in range(2):
        ps = psum.tile([128, n], F32, tag="ps_big")
        for k2 in range(2):
            nc.tensor.matmul(ps, smw[:, 6 + k2, m2 * 128 : (m2 + 1) * 128],
                             xT[:, k2], start=(k2 == 0), stop=(k2 == 1))
        nc.vector.tensor_scalar_add(vT[:, m2], ps, bias("vb", m2))
    # a.T = x.T + ow @ v.T (+ob)
    aT = work.tile([128, 2, n], F32, tag="aT")
    sq = work.tile([128, 2, n], F32, tag="sq")
    for m2 in range(2):
        ps = psum.tile([128, n], F32, tag="ps_big")
        for k2 in range(2):
            nc.tensor.matmul(ps, smw[:, 8 + k2, m2 * 128 : (m2 + 1) * 128],
                             vT[:, k2], start=(k2 == 0), stop=(k2 == 1))
        nc.vector.scalar_tensor_tensor(out=aT[:, m2], in0=ps,
                                       scalar=bias("ob", m2),
                                       in1=xT32[:, m2], op0=AT.add, op1=AT.add)
        nc.scalar.activation(out=sq[:, m2], in_=aT[:, m2],
                             func=mybir.ActivationFunctionType.Square)
    # layernorm stats via ones-matmul over the 256-partition dim
    s1 = psum.tile([1, n], F32, tag="ps_stat")
    s2 = psum.tile([1, n], F32, tag="ps_stat")
    for m2 in range(2):
        nc.tensor.matmul(s1, ones_col, aT[:, m2], start=(m2 == 0), stop=(m2 == 1))
        nc.tensor.matmul(s2, ones_col, sq[:, m2], start=(m2 == 0), stop=(m2 == 1))
    mu = work.tile([1, n], F32, tag="mu")
    nc.vector.tensor_scalar_mul(mu, s1, 1.0 / EMB)
    ex2 = work.tile([1, n], F32, tag="ex2")
    nc.vector.tensor_scalar_mul(ex2, s2, 1.0 / EMB)
    var = work.tile([1, n], F32, tag="var")
    nc.vector.tensor_tensor(out=var, in0=mu, in1=mu, op=AT.mult)
    nc.vector.tensor_tensor(out=var, in0=ex2, in1=var, op=AT.subtract)
    std = work.tile([1, n], F32, tag="std")
    nc.scalar.activation(out=std, in_=var,
                         func=mybir.ActivationFunctionType.Sqrt,
                         bias=eps_sb[:, 0:1], scale=1.0)
    rstd = work.tile([1, n], F32, tag="rstd")
    nc.vector.reciprocal(rstd, std)
    nmur = work.tile([1, n], F32, tag="nmur")
    nc.vector.tensor_tensor(out=nmur, in0=mu, in1=rstd, op=AT.mult)
    nc.vector.tensor_scalar_mul(nmur, nmur, -1.0)
    bc_r = psum.tile([128, n], F32, tag="ps_stat")
    nc.tensor.matmul(bc_r, ones_row, rstd, start=True, stop=True)
    bc_m = psum.tile([128, n], F32, tag="ps_stat")
    nc.tensor.matmul(bc_m, ones_row, nmur, start=True, stop=True)
    hT = work.tile([128, 2, n], BF16, tag="hT")
    tmp = work.tile([128, n], F32, tag="lntmp")
    for m2 in range(2):
        nc.vector.tensor_tensor(out=tmp, in0=aT[:, m2], in1=bc_r, op=AT.mult)
        nc.vector.tensor_tensor(out=tmp, in0=tmp, in1=bc_m, op=AT.add)
        nc.vector.tensor_scalar(out=hT[:, m2], in0=tmp,
                                scalar1=bias("lng", m2), scalar2=bias("lnb", m2),
                                op0=AT.mult, op1=AT.add)

    # p1 -> relu -> p2 accumulate; p2 resident (one 32-128KB/partition DMA),
    # p1 resident when it fits alongside, else streamed in m-groups
    nj = msz // 128
    outps = []
    for j in range(nj):
        opj = psout.tile([128, n], F32, tag=f"ps_out{j}", name=f"outps_{j}")
        outps.append(opj)
    ngrp = (P1 // 128) // MGRP
    for g in range(ngrp):
        p1t = wstream.tile([128, 2, MGRP * 128], BF16, tag="p1s",
                           name=f"p1t_{c}_{ji}_{g}")
        nc.sync.dma_start(out=p1t, in_=d[f"p1wT{e}"][g])
        p2t = wstream.tile([128, MGRP, msz], BF16, tag="p2s",
                           name=f"p2t_{c}_{ji}_{g}")
        nc.sync.dma_start(
            out=p2t, in_=d[f"p2w_c{c}_j{ji}"][:, g * MGRP : (g + 1) * MGRP, :])
        for mi in range(MGRP):
            m = g * MGRP + mi
            l1ps = psum.tile([128, n], F32, tag="ps_big")
            nc.tensor.matmul(l1ps, p1t[:, 0, mi * 128 : (mi + 1) * 128],
                             hT[:, 0], start=True, stop=False)
            nc.tensor.matmul(l1ps, p1t[:, 1, mi * 128 : (mi + 1) * 128],
                             hT[:, 1], start=False, stop=True)
            l1sb = l1pool.tile([128, n], BF16, tag="l1sb")
            if m % 2 == 0:
                nc.vector.tensor_scalar(out=l1sb, in0=l1ps,
                                        scalar1=fb[:, m : m + 1],
                                        scalar2=0.0, op0=AT.add, op1=AT.max)
            else:
                nc.scalar.activation(out=l1sb, in_=l1ps,
                                     func=mybir.ActivationFunctionType.Relu,
                                     bias=fb[:, m : m + 1], scale=1.0)
            for j in range(nj):
                nc.tensor.matmul(outps[j],
                                 p2t[:, mi, j * 128 : (j + 1) * 128], l1sb,
                                 start=(m == 0), stop=(m == P1 // 128 - 1))
    for j in range(nj):
        gj = m0 // 128 + j
        ot = work.tile([128, n], F32, tag="otile")
        nc.scalar.activation(out=ot, in_=outps[j],
                             func=mybir.ActivationFunctionType.Tanh,
                             bias=fb[:, 128 + gj : 128 + gj + 1], scale=1.0)
        nc.sync.dma_start(out=d[f"out_c{c}_j{ji}"][j * 128 : (j + 1) * 128, :],
                          in_=ot)


def kernel(**inputs):
    inp = {k: np.asarray(v) for k, v in inputs.items()}
    feat = _patchify(inp["image"])

    hr = np.maximum(feat @ inp["rw1"].T + inp["rb1"], 0.0)
    logits = hr @ inp["rw2"].T + inp["rb2"]
    eid_host = logits.argmax(1)
    counts = np.bincount(eid_host, minlength=E)
    order = np.argsort(eid_host, kind="stable")
    jobs = _plan(counts)

    nc = _build(jobs)

    im = {}
    featT = np.ascontiguousarray(feat.T)  # [768, 1024]
    fr = featT.reshape(KC_FEAT, 128, NPATCH)
    for c in range(NCORES):
        im[f"featr{c}"] = np.ascontiguousarray(
            fr[:, :, c * 128 : (c + 1) * 128].transpose(1, 0, 2))
    rwp = np.zeros((128, KC_FEAT * EMB + 2 * E), np.float32)
    rw1T = np.ascontiguousarray(inp["rw1"].T).reshape(KC_FEAT, 128, EMB)
    for k in range(KC_FEAT):
        rwp[:, k * EMB : (k + 1) * EMB] = rw1T[k]
    rw2T = np.ascontiguousarray(inp["rw2"].T).reshape(2, 128, E)
    for k2 in range(2):
        rwp[:, KC_FEAT * EMB + k2 * E : KC_FEAT * EMB + (k2 + 1) * E] = rw2T[k2]
    im["rwpack"] = rwp
    im["rb1"] = np.ascontiguousarray(
        inp["rb1"].reshape(2, 128).T).astype(np.float32)
    im["rb2row"] = inp["rb2"].reshape(1, E).astype(np.float32)

    featS = featT[:, order]  # sorted by expert
    p2w = [inp["p2_w0"], inp["p2_w1"], inp["p2_w2"], inp["p2_w3"]]
    p2b = [inp["p2_b0"], inp["p2_b1"], inp["p2_b2"], inp["p2_b3"]]
    experts_used = sorted({jb["e"] for cj in jobs for jb in cj})
    for e in experts_used:
        smw = np.zeros((128, 10, EMB), np.float32)
        pewT = np.ascontiguousarray(inp["pe_w"][e].T).reshape(KC_FEAT, 128, EMB)
        for k in range(KC_FEAT):
            smw[:, k] = pewT[k]
        wvT = np.ascontiguousarray(inp["inw"][e][2 * EMB :].T).reshape(2, 128, EMB)
        owT = np.ascontiguousarray(inp["ow"][e].T).reshape(2, 128, EMB)
        for k2 in range(2):
            smw[:, 6 + k2] = wvT[k2]
            smw[:, 8 + k2] = owT[k2]
        im[f"smw{e}"] = smw.astype(BF)
        im[f"p1wT{e}"] = np.ascontiguousarray(
            inp["p1_w"][e].T.reshape(2, 128, P1 // (MGRP * 128), MGRP * 128)
            .transpose(2, 1, 0, 3)).astype(BF)
        NJ = OUT_E[e] // 128
        fbx = np.zeros((128, 128 + NJ + 10), np.float32)
        fbx[:, 0:128] = inp["p1_b"][e].reshape(128, 128).T
        fbx[:, 128 : 128 + NJ] = p2b[e].reshape(NJ, 128).T
        b10 = fbx[:, 128 + NJ :]
        b10[:, 0:2] = inp["pe_b"][e].reshape(2, 128).T
        b10[:, 2:4] = inp["inb"][e][2 * EMB :].reshape(2, 128).T
        b10[:, 4:6] = inp["ob"][e].reshape(2, 128).T
        b10[:, 6:8] = inp["ln_g"][e].reshape(2, 128).T
        b10[:, 8:10] = inp["ln_b"][e].reshape(2, 128).T
        im[f"fb{e}"] = fbx

    for c in range(NCORES):
        for ji, jb in enumerate(jobs[c]):
            e, p0, n = jb["e"], jb["p0"], jb["n_pad"]
            sl = featS[:, p0 : p0 + n]
            if sl.shape[1] < n:
                sl = np.pad(sl, ((0, 0), (0, n - sl.shape[1])))
            im[f"feat_c{c}_j{ji}"] = np.ascontiguousarray(
                sl.reshape(KC_FEAT, 128, n).transpose(1, 0, 2)).astype(BF)
            # p2 weights: [krow 128, mchunk 128, msz] contiguous
            w = p2w[e].T[:, jb["m0"] : jb["m0"] + jb["msz"]]  # [16384, msz]
            im[f"p2w_c{c}_j{ji}"] = np.ascontiguousarray(
                w.reshape(128, 128, jb["msz"]).transpose(1, 0, 2)).astype(BF)

    in_maps = [im for _ in range(NCORES)]
    r = run_bass_kernel_spmd(nc, in_maps, list(range(NCORES)))
    kernel._last_results = r

    probs = np.concatenate([r.results[c]["probs"] for c in range(NCORES)], 0)
    eid = np.concatenate(
        [r.results[c]["eid"][:, 0] for c in range(NCORES)], 0).astype(np.int32)

    ch_base = np.concatenate([[0], np.cumsum(CL)])
    latents = np.zeros((NPATCH, sum(CL), PS // 4, PS // 4), np.float32)
    for c in range(NCORES):
        for ji, jb in enumerate(jobs[c]):
            arr = r.results[c][f"out_c{c}_j{ji}"]  # [msz, n_pad]
            nr = jb["n_real"]
            ids = order[jb["p0"] : jb["p0"] + nr]
            blk = arr[:, :nr].T.reshape(nr, jb["msz"] // 16, 4, 4)
            cb = ch_base[jb["e"]] + jb["m0"] // 16
            latents[ids, cb : cb + jb["msz"] // 16] = blk
    return probs, eid, latents


# revision 15
# speedup vs baseline: 1.0919x; 1.0919x over previous
"""MoE patch encoder on 8 Trainium2 NeuronCores.

Strategy:
- Host: patchify, run the (tiny) router in fp32 to build a dispatch plan:
  patches sorted by expert, cores assigned (expert, patch-range, out-range)
  jobs balancing HBM bytes and PE flops. Device re-computes the router in
  fp32 for the probs/expert_id outputs.
- Device (one SPMD program, per-core arms via an If tree on partition id):
  each core routes its 128 patches (fp32) and runs its expert jobs in bf16
  (weights cast on host): x -> attn(v) -> layernorm -> p1(relu) -> p2(tanh),
  all in transposed [feature, patch] layout so biases are per-partition.
- Host: scatter job outputs back to the full [1024, 120, 4, 4] latents.
"""
import sys
import os
import numpy as np

for _p in ("/opt/trn_rl_repo", os.path.dirname(os.path.abspath(__file__))):
    if _p not in sys.path:
        sys.path.insert(0, _p)

import ml_dtypes
import concourse.bass as bass
import concourse.mybir as mybir
import concourse.tile as tile
from concourse.bass_utils import run_bass_kernel_spmd
from contextlib import ExitStack

F32 = mybir.dt.float32
BF16 = mybir.dt.bfloat16
I32 = mybir.dt.int32
U32 = mybir.dt.uint32
BF = ml_dtypes.bfloat16

PS = 16
EMB = 256
E = 4
CL = [8, 16, 32, 64]
OUT_E = [c * 16 for c in CL]  # [128, 256, 512, 1024]
P1 = 16384
H = W = 512
NPATCH = (H // PS) * (W // PS)  # 1024
NCORES = 8
FEAT = 3 * PS * PS  # 768
KC_FEAT = FEAT // 128  # 6
MGRP = 16  # p1/p2 m-chunks per DMA group

NEG_BIG = -1.0e30


def _cap_sync_waits(nc, max_waits=1):
    """walrus here rejects >N waits per instruction; split onto nops."""
    def fix_block(bb):
        new_insts = []
        for inst in bb.instructions:
            si = getattr(inst, "sync_info", None)
            if si is not None and si.on_wait and len(si.on_wait) > max_waits:
                waits = list(si.on_wait)
                extra, keep = waits[:-max_waits], waits[-max_waits:]
                for i in range(0, len(extra), max_waits):
                    chunk = extra[i : i + max_waits]
                    nop = mybir.InstNoOp(
                        name=f"{inst.name}-ws{i}",
                        sync_info=mybir.SyncInfo(on_wait=chunk, on_update=[]),
                        engine=inst.engine,
                        bass_nofuse=True,
                    )
                    new_insts.append(nop)
                si.on_wait = keep
            new_insts.append(inst)
        bb.instructions = new_insts

    def walk(blocks):
        for b in blocks:
            if hasattr(b, "instructions"):
                fix_block(b)
            inner = getattr(b, "blocks", None)
            if inner:
                walk(inner)

    for f in nc.m.functions:
        walk(f.blocks)


def _patchify(image):
    img = np.asarray(image, np.float32)[0]
    gh, gw = H // PS, W // PS
    return (
        img.reshape(3, gh, PS, gw, PS)
        .transpose(1, 3, 0, 2, 4)
        .reshape(gh * gw, FEAT)
    )


def _split_even(total, parts):
    q, r = divmod(total, parts)
    return [q + (1 if i < r else 0) for i in range(parts)]


def _expert_split(e, n_e, g):
    """Best (per-core time, per-core job lists) for expert e on g cores."""
    NJ = OUT_E[e] // 128
    best = None
    for mm in range(1, g + 1):
        if g % mm or mm > NJ:
            continue
        pm = g // mm
        jgrp = _split_even(NJ, mm)
        nblk = -(-n_e // pm)
        pjobs = -(-nblk // 512)
        cores = []
        tmax = 0.0
        for mi in range(mm):
            j0 = sum(jgrp[:mi])
            gj = jgrp[mi]
            mjobs = -(-gj // 4)  # psum limit: 4 j-chunks per job
            jsz = _split_even(gj, mjobs)
            for pi in range(pm):
                p_off = pi * nblk
                n_here = max(0, min(n_e - p_off, nblk))
                jl = []
                for pj in range(pjobs):
                    po = p_off + pj * 512
                    nr = max(0, min(n_here - pj * 512, 512))
                    if nr == 0:
                        continue
                    jj = j0
                    for js in jsz:
                        jl.append((po, nr, jj, js))
                        jj += js
                cores.append(jl)
                b = sum(P1 * EMB * 2 + P1 * (js_ * 128) * 2
                        for (_, _, _, js_) in jl) + n_here * FEAT * 2 + 2.0e6
                f = sum(nr_ * 2.0 * (EMB * P1 + P1 * 128 * js_ + FEAT * EMB
                                     + 2 * EMB * EMB)
                        for (_, nr_, _, js_) in jl)
                t = max(b / 340e3, f / 70e3) + 8e3 + len(jl) * 4e3
                tmax = max(tmax, t)
        if best is None or tmax < best[0]:
            best = (tmax, cores)
    return best


def _plan(counts):
    """Assign 8 cores to (expert, patch-range, out-range) jobs."""
    seg = np.concatenate([[0], np.cumsum(counts)])
    active = [e for e in range(E) if counts[e] > 0]

    def compositions(total, parts):
        if parts == 1:
            yield (total,)
            return
        for first in range(1, total - parts + 2):
            for rest in compositions(total - first, parts - 1):
                yield (first,) + rest

    best = None
    for alloc in compositions(NCORES, len(active)):
        tmax = 0.0
        detail = []
        for e, g in zip(active, alloc):
            r = _expert_split(e, int(counts[e]), g)
            if r is None:
                tmax = float("inf")
                break
            tmax = max(tmax, r[0])
            detail.append((e, r[1]))
        if best is None or tmax < best[0]:
            best = (tmax, detail)

    jobs = [[] for _ in range(NCORES)]
    core = 0
    for e, cores in best[1]:
        for jl in cores:
            for (po, nr, j0, njc) in jl:
                n_pad = min(512, max(16, -(-nr // 8) * 8))
                jobs[core].append(dict(
                    e=e, p0=int(seg[e] + po), n_real=int(nr), n_pad=int(n_pad),
                    m0=int(j0 * 128), msz=int(njc * 128)))
            core += 1
    return jobs


# bias10 column map (per expert, appended to fb array):
#   0,1: pe_b k2=0,1   2,3: vb   4,5: ob   6,7: ln_g   8,9: ln_b
BIAS_COLS = {"peb": 0, "vb": 2, "ob": 4, "lng": 6, "lnb": 8}


def _build(jobs):
    nc = bass.Bass()
    d = {}

    def din(name, shape, dt):
        d[name] = nc.dram_tensor(name, list(shape), dt, kind="ExternalInput")
        return d[name]

    def dout(name, shape, dt):
        d[name] = nc.dram_tensor(name, list(shape), dt, kind="ExternalOutput")
        return d[name]

    for c in range(NCORES):
        din(f"featr{c}", (128, KC_FEAT, 128), F32)
    din("rwpack", (128, KC_FEAT * EMB + 2 * E), F32)
    din("rb1", (128, 2), F32)
    din("rb2row", (1, E), F32)

    experts_used = sorted({jb["e"] for cj in jobs for jb in cj})
    for e in experts_used:
        din(f"smw{e}", (128, 10, EMB), BF16)
        din(f"p1wT{e}", (P1 // (MGRP * 128), 128, 2, MGRP * 128), BF16)
        din(f"fb{e}", (128, 128 + OUT_E[e] // 128 + 10), F32)
    for c in range(NCORES):
        for ji, jb in enumerate(jobs[c]):
            din(f"feat_c{c}_j{ji}", (128, KC_FEAT, jb["n_pad"]), BF16)
            din(f"p2w_c{c}_j{ji}", (128, 128, jb["msz"]), BF16)

    dout("probs", (128, E), F32)
    dout("eid", (128, 1), I32)
    for c in range(NCORES):
        for ji, jb in enumerate(jobs[c]):
            dout(f"out_c{c}_j{ji}", (jb["msz"], jb["n_pad"]), F32)

    with tile.TileContext(nc) as tc, ExitStack() as ctx:
        const = ctx.enter_context(tc.tile_pool(name="const", bufs=1))
        work = ctx.enter_context(tc.tile_pool(name="work", bufs=2))
        wstream = ctx.enter_context(tc.tile_pool(name="wstream", bufs=2))
        wres = ctx.enter_context(tc.tile_pool(name="wres", bufs=1))
        l1pool = ctx.enter_context(tc.tile_pool(name="l1pool", bufs=3))
        psum = ctx.enter_context(tc.tile_pool(name="psum", bufs=2, space="PSUM"))
        psout = ctx.enter_context(tc.tile_pool(name="psout", bufs=1, space="PSUM"))

        ones_col = const.tile([128, 1], F32)
        nc.vector.memset(ones_col, 1.0)
        ones_row = const.tile([1, 128], F32)
        nc.vector.memset(ones_row, 1.0)
        eps_sb = const.tile([1, 1], F32)
        nc.vector.memset(eps_sb, 1.0e-5)

        pid = nc.partition_id()

        def emit_arm(c):
            with tc.high_priority():
                _emit_router(nc, tc, d, c, work, psum, ones_row)
            loaded = {}
            for ji, jb in enumerate(jobs[c]):
                e = jb["e"]
                if e not in loaded:
                    smw = work.tile([128, 10, EMB], BF16, name=f"smw_{c}_{e}",
                                    tag="smw")
                    nc.sync.dma_start(out=smw, in_=d[f"smw{e}"][:])
                    fb = work.tile([128, 128 + OUT_E[e] // 128 + 10], F32,
                                   name=f"fb_{c}_{e}", tag="fb")
                    nc.sync.dma_start(out=fb, in_=d[f"fb{e}"][:])
                    loaded[e] = (smw, fb)
                smw, fb = loaded[e]
                _emit_job(nc, tc, d, c, ji, jb, smw, fb, work, wstream, wres,
                          l1pool, psum, psout, ones_col, ones_row, eps_sb)

        def emit_tree(lo, hi):
            if hi - lo == 1:
                emit_arm(lo)
                return
            mid = (lo + hi) // 2
            with tc.If(pid < mid) as cmp:
                emit_tree(lo, mid)
            with cmp.Else():
                emit_tree(mid, hi)

        emit_tree(0, NCORES)

    _cap_sync_waits(nc, max_waits=1)
    return nc


def _emit_router(nc, tc, d, c, work, psum, ones_row):
    featr = work.tile([128, KC_FEAT, 128], F32, tag="featr")
    nc.sync.dma_start(out=featr, in_=d[f"featr{c}"][:])
    rwp = work.tile([128, KC_FEAT * EMB + 2 * E], F32, tag="rwp")
    nc.sync.dma_start(out=rwp, in_=d["rwpack"][:])
    rb1_sb = work.tile([128, 2], F32, tag="rb1sb")
    nc.sync.dma_start(out=rb1_sb, in_=d["rb1"][:])
    rb2_sb = work.tile([1, E], F32, tag="rb2sb")
    nc.sync.dma_start(out=rb2_sb, in_=d["rb2row"][:])

    hrT = work.tile([128, 2, 128], F32, tag="hrT")
    for m2 in range(2):
        ps = psum.tile([128, 128], F32, tag="ps_big")
        for k in range(KC_FEAT):
            nc.tensor.matmul(
                ps, rwp[:, k * EMB + m2 * 128 : k * EMB + (m2 + 1) * 128],
                featr[:, k], start=(k == 0), stop=(k == KC_FEAT - 1))
        nc.vector.tensor_scalar(out=hrT[:, m2], in0=ps,
                                scalar1=rb1_sb[:, m2 : m2 + 1], scalar2=0.0,
                                op0=mybir.AluOpType.add, op1=mybir.AluOpType.max)
    lg_ps = psum.tile([128, E], F32, tag="ps_big")
    for k2 in range(2):
        nc.tensor.matmul(
            lg_ps, hrT[:, k2],
            rwp[:, KC_FEAT * EMB + k2 * E : KC_FEAT * EMB + (k2 + 1) * E],
            start=(k2 == 0), stop=False)
    nc.tensor.matmul(lg_ps, ones_row, rb2_sb, start=False, stop=True)

    lg = work.tile([128, 8], F32, tag="lg")
    nc.vector.memset(lg, NEG_BIG)
    nc.vector.tensor_copy(lg[:, 0:E], lg_ps)
    mx = work.tile([128, 1], F32, tag="mx")
    nc.vector.tensor_reduce(out=mx, in_=lg[:, 0:E], axis=mybir.AxisListType.X,
                            op=mybir.AluOpType.max)
    negmx = work.tile([128, 1], F32, tag="negmx")
    nc.vector.tensor_scalar_mul(negmx, mx, -1.0)
    expt = work.tile([128, E], F32, tag="expt")
    sume = work.tile([128, 1], F32, tag="sume")
    nc.scalar.activation(out=expt, in_=lg[:, 0:E],
                         func=mybir.ActivationFunctionType.Exp,
                         bias=negmx[:, 0:1], scale=1.0, accum_out=sume)
    rec = work.tile([128, 1], F32, tag="rec")
    nc.vector.reciprocal(rec, sume)
    probs = work.tile([128, E], F32, tag="probs")
    nc.vector.tensor_scalar_mul(probs, expt, rec[:, 0:1])
    nc.sync.dma_start(out=d["probs"][:], in_=probs)

    top8 = work.tile([128, 8], F32, tag="top8")
    nc.vector.max(top8, lg)
    idx8 = work.tile([128, 8], U32, tag="idx8")
    nc.vector.max_index(idx8, top8, lg)
    eid = work.tile([128, 1], I32, tag="eid")
    nc.vector.tensor_copy(eid, idx8[:, 0:1])
    nc.sync.dma_start(out=d["eid"][:], in_=eid)


def _emit_job(nc, tc, d, c, ji, jb, smw, fb, work, wstream, wres, l1pool, psum,
              psout, ones_col, ones_row, eps_sb):
    e, n, m0, msz = jb["e"], jb["n_pad"], jb["m0"], jb["msz"]
    AT = mybir.AluOpType
    NJ_E = OUT_E[e] // 128

    def bias(nm, m2):
        col = 128 + NJ_E + BIAS_COLS[nm] + m2
        return fb[:, col : col + 1]

    feats = work.tile([128, KC_FEAT, n], BF16, tag="feats")
    nc.sync.dma_start(out=feats, in_=d[f"feat_c{c}_j{ji}"][:])

    # x.T = pe_w @ feat.T (+peb); bf16 for matmuls, f32 for the residual
    xT = work.tile([128, 2, n], BF16, tag="xT")
    xT32 = work.tile([128, 2, n], F32, tag="xT32")
    for m2 in range(2):
        ps = psum.tile([128, n], F32, tag="ps_big")
        for k in range(KC_FEAT):
            nc.tensor.matmul(ps, smw[:, k, m2 * 128 : (m2 + 1) * 128],
                             feats[:, k], start=(k == 0), stop=(k == KC_FEAT - 1))
        nc.vector.tensor_scalar_add(xT[:, m2], ps, bias("peb", m2))
        nc.vector.tensor_scalar_add(xT32[:, m2], ps, bias("peb", m2))
    # v.T = wv @ x.T (+vb)
    vT = work.tile([128, 2, n], BF16, tag="vT")
    for m2 in range(2):
        ps = psum.tile([128, n], F32, tag="ps_big")
        for k2 in range(2):
            nc.tensor.matmul(ps, smw[:, 6 + k2, m2 * 128 : (m2 + 1) * 128],
                             xT[:, k2], start=(k2 == 0), stop=(k2 == 1))
        nc.vector.tensor_scalar_add(vT[:, m2], ps, bias("vb", m2))
    # a.T = x.T + ow @ v.T (+ob)
    aT = work.tile([128, 2, n], F32, tag="aT")
    sq = work.tile([128, 2, n], F32, tag="sq")
    for m2 in range(2):
        ps = psum.tile([128, n], F32, tag="ps_big")
        for k2 in range(2):
            nc.tensor.matmul(ps, smw[:, 8 + k2, m2 * 128 : (m2 + 1) * 128],
                             vT[:, k2], start=(k2 == 0), stop=(k2 == 1))
        nc.vector.scalar_tensor_tensor(out=aT[:, m2], in0=ps,
                                       scalar=bias("ob", m2),
                                       in1=xT32[:, m2], op0=AT.add, op1=AT.add)
        nc.scalar.activation(out=sq[:, m2], in_=aT[:, m2],
                             func=mybir.ActivationFunctionType.Square)
    # layernorm stats via ones-matmul over the 256-partition dim
    s1 = psum.tile([1, n], F32, tag="ps_stat")
    s2 = psum.tile([1, n], F32, tag="ps_stat")
    for m2 in range(2):
        nc.tensor.matmul(s1, ones_col, aT[:, m2], start=(m2 == 0), stop=(m2 == 1))
        nc.tensor.matmul(s2, ones_col, sq[:, m2], start=(m2 == 0), stop=(m2 == 1))
    mu = work.tile([1, n], F32, tag="mu")
    nc.vector.tensor_scalar_mul(mu, s1, 1.0 / EMB)
    ex2 = work.tile([1, n], F32, tag="ex2")
    nc.vector.tensor_scalar_mul(ex2, s2, 1.0 / EMB)
    var = work.tile([1, n], F32, tag="var")
    nc.vector.tensor_tensor(out=var, in0=mu, in1=mu, op=AT.mult)
    nc.vector.tensor_tensor(out=var, in0=ex2, in1=var, op=AT.subtract)
    std = work.tile([1, n], F32, tag="std")
    nc.scalar.activation(out=std, in_=var,
                         func=mybir.ActivationFunctionType.Sqrt,
                         bias=eps_sb[:, 0:1], scale=1.0)
    rstd = work.tile([1, n], F32, tag="rstd")
    nc.vector.reciprocal(rstd, std)
    nmur = work.tile([1, n], F32, tag="nmur")
    nc.vector.tensor_tensor(out=nmur, in0=mu, in1=rstd, op=AT.mult)
    nc.vector.tensor_scalar_mul(nmur, nmur, -1.0)
    bc_r = psum.tile([128, n], F32, tag="ps_stat")
    nc.tensor.matmul(bc_r, ones_row, rstd, start=True, stop=True)
    bc_m = psum.tile([128, n], F32, tag="ps_stat")
    nc.tensor.matmul(bc_m, ones_row, nmur, start=True, stop=True)
    hT = work.tile([128, 2, n], BF16, tag="hT")
    tmp = work.tile([128, n], F32, tag="lntmp")
    for m2 in range(2):
        nc.vector.tensor_tensor(out=tmp, in0=aT[:, m2], in1=bc_r, op=AT.mult)
        nc.vector.tensor_tensor(out=tmp, in0=tmp, in1=bc_m, op=AT.add)
        nc.vector.tensor_scalar(out=hT[:, m2], in0=tmp,
                                scalar1=bias("lng", m2), scalar2=bias("lnb", m2),
                                op0=AT.mult, op1=AT.add)

    # p1 -> relu -> p2 accumulate; p2 resident (one 32-128KB/partition DMA),
    # p1 resident when it fits alongside, else streamed in m-groups
    nj = msz // 128
    outps = []
    for j in range(nj):
        opj = psout.tile([128, n], F32, tag=f"ps_out{j}", name=f"outps_{j}")
        outps.append(opj)
    ngrp = (P1 // 128) // MGRP
    for g in range(ngrp):
        p1t = wstream.tile([128, 2, MGRP * 128], BF16, tag="p1s",
                           name=f"p1t_{c}_{ji}_{g}")
        nc.sync.dma_start(out=p1t, in_=d[f"p1wT{e}"][g])
        p2t = wstream.tile([128, MGRP, msz], BF16, tag="p2s",
                           name=f"p2t_{c}_{ji}_{g}")
        nc.sync.dma_start(
            out=p2t, in_=d[f"p2w_c{c}_j{ji}"][:, g * MGRP : (g + 1) * MGRP, :])
        for mi in range(MGRP):
            m = g * MGRP + mi
            l1ps = psum.tile([128, n], F32, tag="ps_big")
            nc.tensor.matmul(l1ps, p1t[:, 0, mi * 128 : (mi + 1) * 128],
                             hT[:, 0], start=True, stop=False)
            nc.tensor.matmul(l1ps, p1t[:, 1, mi * 128 : (mi + 1) * 128],
                             hT[:, 1], start=False, stop=True)
            l1sb = l1pool.tile([128, n], BF16, tag="l1sb")
            if m % 2 == 0:
                nc.vector.tensor_scalar(out=l1sb, in0=l1ps,
                                        scalar1=fb[:, m : m + 1],
                                        scalar2=0.0, op0=AT.add, op1=AT.max)
            else:
                nc.scalar.activation(out=l1sb, in_=l1ps,
                                     func=mybir.ActivationFunctionType.Relu,
                                     bias=fb[:, m : m + 1], scale=1.0)
            for j in range(nj):
                nc.tensor.matmul(outps[j],
                                 p2t[:, mi, j * 128 : (j + 1) * 128], l1sb,
                                 start=(m == 0), stop=(m == P1 // 128 - 1))
    for j in range(nj):
        gj = m0 // 128 + j
        ot = work.tile([128, n], F32, tag="otile")
        nc.scalar.activation(out=ot, in_=outps[j],
                             func=mybir.ActivationFunctionType.Tanh,
                             bias=fb[:, 128 + gj : 128 + gj + 1], scale=1.0)
        nc.sync.dma_start(out=d[f"out_c{c}_j{ji}"][j * 128 : (j + 1) * 128, :],
                          in_=ot)


def kernel(**inputs):
    inp = {k: np.asarray(v) for k, v in inputs.items()}
    feat = _patchify(inp["image"])

    hr = np.maximum(feat @ inp["rw1"].T + inp["rb1"], 0.0)
    logits = hr @ inp["rw2"].T + inp["rb2"]
    eid_host = logits.argmax(1)
    counts = np.bincount(eid_host, minlength=E)
    order = np.argsort(eid_host, kind="stable")
    jobs = _plan(counts)

    nc = _build(jobs)

    im = {}
    featT = np.ascontiguousarray(feat.T)  # [768, 1024]
    fr = featT.reshape(KC_FEAT, 128, NPATCH)
    for c in range(NCORES):
        im[f"featr{c}"] = np.ascontiguousarray(
            fr[:, :, c * 128 : (c + 1) * 128].transpose(1, 0, 2))
    rwp = np.zeros((128, KC_FEAT * EMB + 2 * E), np.float32)
    rw1T = np.ascontiguousarray(inp["rw1"].T).reshape(KC_FEAT, 128, EMB)
    for k in range(KC_FEAT):
        rwp[:, k * EMB : (k + 1) * EMB] = rw1T[k]
    rw2T = np.ascontiguousarray(inp["rw2"].T).reshape(2, 128, E)
    for k2 in range(2):
        rwp[:, KC_FEAT * EMB + k2 * E : KC_FEAT * EMB + (k2 + 1) * E] = rw2T[k2]
    im["rwpack"] = rwp
    im["rb1"] = np.ascontiguousarray(
        inp["rb1"].reshape(2, 128).T).astype(np.float32)
    im["rb2row"] = inp["rb2"].reshape(1, E).astype(np.float32)

    featS = featT[:, order]  # sorted by expert
    p2w = [inp["p2_w0"], inp["p2_w1"], inp["p2_w2"], inp["p2_w3"]]
    p2b = [inp["p2_b0"], inp["p2_b1"], inp["p2_b2"], inp["p2_b3"]]
    experts_used = sorted({jb["e"] for cj in jobs for jb in cj})
    for e in experts_used:
        smw = np.zeros((128, 10, EMB), np.float32)
        pewT = np.ascontiguousarray(inp["pe_w"][e].T).reshape(KC_FEAT, 128, EMB)
        for k in range(KC_FEAT):
            smw[:, k] = pewT[k]
        wvT = np.ascontiguousarray(inp["inw"][e][2 * EMB :].T).reshape(2, 128, EMB)
        owT = np.ascontiguousarray(inp["ow"][e].T).reshape(2, 128, EMB)
        for k2 in range(2):
            smw[:, 6 + k2] = wvT[k2]
            smw[:, 8 + k2] = owT[k2]
        im[f"smw{e}"] = smw.astype(BF)
        im[f"p1wT{e}"] = np.ascontiguousarray(
            inp["p1_w"][e].T.reshape(2, 128, P1 // (MGRP * 128), MGRP * 128)
            .transpose(2, 1, 0, 3)).astype(BF)
        NJ = OUT_E[e] // 128
        fbx = np.zeros((128, 128 + NJ + 10), np.float32)
        fbx[:, 0:128] = inp["p1_b"][e].reshape(128, 128).T
        fbx[:, 128 : 128 + NJ] = p2b[e].reshape(NJ, 128).T
        b10 = fbx[:, 128 + NJ :]
        b10[:, 0:2] = inp["pe_b"][e].reshape(2, 128).T
        b10[:, 2:4] = inp["inb"][e][2 * EMB :].reshape(2, 128).T
        b10[:, 4:6] = inp["ob"][e].reshape(2, 128).T
        b10[:, 6:8] = inp["ln_g"][e].reshape(2, 128).T
        b10[:, 8:10] = inp["ln_b"][e].reshape(2, 128).T
        im[f"fb{e}"] = fbx

    for c in range(NCORES):
        for ji, jb in enumerate(jobs[c]):
            e, p0, n = jb["e"], jb["p0"], jb["n_pad"]
            sl = featS[:, p0 : p0 + n]
            if sl.shape[1] < n:
                sl = np.pad(sl, ((0, 0), (0, n - sl.shape[1])))
            im[f"feat_c{c}_j{ji}"] = np.ascontiguousarray(
                sl.reshape(KC_FEAT, 128, n).transpose(1, 0, 2)).astype(BF)
            # p2 weights: [krow 128, mchunk 128, msz] contiguous
            w = p2w[e].T[:, jb["m0"] : jb["m0"] + jb["msz"]]  # [16384, msz]
            im[f"p2w_c{c}_j{ji}"] = np.ascontiguousarray(
                w.reshape(128, 128, jb["msz"]).transpose(1, 0, 2)).astype(BF)

    in_maps = [im for _ in range(NCORES)]
    r = run_bass_kernel_spmd(nc, in_maps, list(range(NCORES)))
    kernel._last_results = r

    probs = np.concatenate([r.results[c]["probs"] for c in range(NCORES)], 0)
    eid = np.concatenate(
        [r.results[c]["eid"][:, 0] for c in range(NCORES)], 0).astype(np.int32)

    ch_base = np.concatenate([[0], np.cumsum(CL)])
    latents = np.zeros((NPATCH, sum(CL), PS // 4, PS // 4), np.float32)
    for c in range(NCORES):
        for ji, jb in enumerate(jobs[c]):
            arr = r.results[c][f"out_c{c}_j{ji}"]  # [msz, n_pad]
            nr = jb["n_real"]
            ids = order[jb["p0"] : jb["p0"] + nr]
            blk = arr[:, :nr].T.reshape(nr, jb["msz"] // 16, 4, 4)
            cb = ch_base[jb["e"]] + jb["m0"] // 16
            latents[ids, cb : cb + jb["msz"] // 16] = blk
    return probs, eid, latents
